# revision 47
# baseline (speedup 1.0000x reference)
"""Multi-head self-attention on 8 Trainium2 NeuronCores.

Sharding: batch (2) x head-groups (4 groups of 4 heads) -> 8 cores.
Per core: x[b] @ wq/wk/wv column slices (256 ch), 4 heads of attention,
row-parallel wo -> partial [2048, 1024] output; host sums the 4 group
partials per batch (the unshard step for row-parallel wo).

v3 dataflow (head-pair packing + PE row tiling, all-bf16 matmuls):
  qT/kT [128, 2*2048] bf16: pair j at cols j*T; head 2j on partitions
        0-63, head 2j+1 on partitions 64-127. Score matmuls contract
        K=64 from base partition 0 / 64 -> they land on PE array tiles
        T0/T8 (64x128 row-tiled mode) and stream CONCURRENTLY (verified
        on HW: the pair overlaps fully), so a head pair's scores cost
        one matmul.
  V     interleaved [2048 t, 4*65+pad] bf16 with a ones column per head
        (PV emits the softmax denominator as PSUM row 64 for free).
  s     PSUM [128 t2, 1024] = both heads' 512-wide t1 quarter,
        ping-pong (s0/s1); ONE exp ACTIVATE [128,1024] per i straight
        off PSUM (scores~N(0,1), no max-subtraction), bf16 out. PV runs
        one i-pair behind exp (software pipeline) so the in-order
        tensor queue never stalls on the exp latency.
  PSUM  s0+s1 (4 banks) + o0+o1 ([128,512] accumulators, 2 banks) +
        p0+p1 (2 spare banks for projection/wo units).
  sched pair-0 Q/K projections run kd-OUTER across all 8 PSUM banks so
        they pipeline with the xT input DMA; V projection, pair-1 Q/K,
        and per-quarter wo units are emitted as fillers inside the
        attention i-loops, deadline-ordered, hiding them in the slack
        between the exp stream (ScalarE, ~1us/iter) and the attention
        matmuls. attnT/wo/y all bf16 (fp32 matmul runs 3-4x slower on
        the PE; bf16 keeps rel err ~7e-3 << 2e-2).
Measured: see test.py.
"""

import sys

sys.path.insert(0, "/opt/trn_rl_repo")

import numpy as np
import ml_dtypes
import concourse.bass as bass
import concourse.mybir as mybir
import concourse.tile as tile
from concourse import bacc
from concourse.bass_utils import run_bass_kernel_spmd

B, T, D = 2, 2048, 1024
NH = 4  # heads per core
HD = 64  # head dim
CH = NH * HD  # 256 channels per core
KD = D // 128  # 8 k-ptiles
CP = CH // 128  # 2 c-ptiles (head pairs)
TP = T // 128  # 16 t-ptiles
QW = 512  # t1 quarter width
NQ = T // QW  # 4 quarters
VW = HD + 1  # 65: v columns + ones column
VROW = NH * VW  # 260

F32 = mybir.dt.float32
EXP = mybir.ActivationFunctionType.Exp
BF16 = mybir.dt.bfloat16

_cached_nc = None


def _wlayout(w):
    """[G*128, C] -> [128, G*C]: host-side relayout matching the SBUF tiles
    so the weight DMAs are fully contiguous."""
    g = w.shape[0] // 128
    return np.ascontiguousarray(
        w.reshape(g, 128, w.shape[1]).transpose(1, 0, 2).reshape(128, -1)
    )


def _build():
    nc = bacc.Bacc(None, target_bir_lowering=False)
    xT = nc.dram_tensor("xT", [128, KD * T], BF16, kind="ExternalInput")
    ones = nc.dram_tensor("ones", [NH * TP, 128], BF16, kind="ExternalInput")
    wq = nc.dram_tensor("wq", [128, KD * CH], BF16, kind="ExternalInput")
    wk = nc.dram_tensor("wk", [128, KD * CH], BF16, kind="ExternalInput")
    wv = nc.dram_tensor("wv", [128, KD * CH], BF16, kind="ExternalInput")
    wo = nc.dram_tensor("wo", [128, CP * D], BF16, kind="ExternalInput")
    y = nc.dram_tensor("y", [T, D], BF16, kind="ExternalOutput")

    with tile.TileContext(nc) as tc:
        with (
            tc.tile_pool(name="sb", bufs=1) as sb,
            tc.tile_pool(name="pexp", bufs=10) as pexp,
            tc.tile_pool(name="small", bufs=3) as small,
            tc.tile_pool(name="ystage", bufs=6) as ystage,
            tc.tile_pool(name="ps_s", bufs=1, space="PSUM") as ps_s,
            tc.tile_pool(name="ps_o", bufs=1, space="PSUM") as ps_o,
            tc.tile_pool(name="ps_p", bufs=1, space="PSUM") as ps_p,
        ):
            wot = sb.tile([128, CP * D], BF16)
            qTt = sb.tile([128, CP * T], BF16)
            kTt = sb.tile([128, CP * T], BF16)
            vt = sb.tile([128, TP * VROW + 64], BF16)
            attnT = sb.tile([128, CP * T], BF16)
            wqt = sb.tile([128, KD * CH], BF16)
            wkt = sb.tile([128, KD * CH], BF16)
            wvt = sb.tile([128, KD * CH], BF16)
            xTt = sb.tile([128, KD * T], BF16)

            # --- input DMAs: wk's second half follows xT so it doesn't
            # delay the xT stream the whole pre-phase is paced by ---
            HW2 = KD * CH // 2
            nc.sync.dma_start(wqt[:], wq[:])
            nc.sync.dma_start(wkt[:, 0:HW2], wk[:, 0:HW2])
            for kd in range(KD):  # per-chunk gating so the pre-phase streams
                nc.sync.dma_start(
                    xTt[:, kd * T : (kd + 1) * T], xT[:, kd * T : (kd + 1) * T]
                )
            nc.sync.dma_start(wkt[:, HW2 : 2 * HW2], wk[:, HW2 : 2 * HW2])
            nc.sync.dma_start(wvt[:], wv[:])
            nc.sync.dma_start(wot[:], wo[:])
            # ones columns of vt: offsets 64 + 65*k, k = 0..NH*TP-1
            nc.sync.dma_start(
                bass.AP(vt.tensor, HD, [[TP * VROW + 64, 128], [VW, NH * TP]]),
                ones.rearrange("k p -> p k"),
            )
            # init the 64-col pad tail (read as junk M-padding by the last
            # head's PV lhsT; must not be uninitialized SBUF)
            nc.sync.dma_start(
                vt[:, TP * VROW : TP * VROW + 64],
                ones.rearrange("k p -> p k"),
            )

            # warm-up: pull the one-time ACT table load (~2.7us for the
            # exp set), custom-DVE reciprocal ucode, and gpsimd broadcast
            # setup into the DMA-wait dead window at t~0 instead of paying
            # them on the first real use mid-stream
            wsc = sb.tile([1, 8], F32)
            nc.vector.memset(wsc[:], 1.0)
            wex = sb.tile([1, 8], BF16)
            nc.scalar.activation(wex[:], wsc[:], EXP, scale=0.125)
            wrc = sb.tile([1, 8], F32)
            nc.vector.reciprocal_approx_fast(wrc[:], wsc[:])
            wbc = sb.tile([64, 8], F32)
            nc.gpsimd.partition_broadcast(wbc[:], wrc[:])

            _palt = [0]

            def proj_qk_unit(cp, dst, wsb, t8):
                # 256-wide (half t-block) so a unit is a ~1us filler piece
                ps = ps_p.tile([128, QW], F32, tag=f"p{_palt[0]}")
                _palt[0] ^= 1
                off = t8 * 256
                for kd in range(KD):
                    nc.tensor.matmul(
                        ps[:, 0:256],
                        wsb[:, kd * CH + cp * 128 : kd * CH + cp * 128 + 128],
                        xTt[:, kd * T + off : kd * T + off + 256],
                        start=(kd == 0),
                        stop=(kd == KD - 1),
                    )
                nc.vector.tensor_copy(
                    dst[:, cp * T + off : cp * T + off + 256], ps[:, 0:256]
                )

            def proj_v_unit(tp):  # one t2 chunk
                ps = ps_p.tile([128, QW], F32, tag=f"p{_palt[0]}")
                _palt[0] ^= 1
                for kd in range(KD):
                    nc.tensor.matmul(
                        ps[:, 0:CH],
                        xTt[:, kd * T + tp * 128 : kd * T + tp * 128 + 128],
                        wvt[:, kd * CH : (kd + 1) * CH],
                        start=(kd == 0),
                        stop=(kd == KD - 1),
                    )
                nc.vector.tensor_copy(
                    bass.AP(
                        vt.tensor,
                        tp * VROW,
                        [[TP * VROW + 64, 128], [VW, NH], [1, HD]],
                    ),
                    ps[:, 0:CH].rearrange("p (h c) -> p h c", h=NH),
                )

            def wo_unit(tp, ob, tail=False):  # y tile [128 t1, 512 d]
                ps = ps_p.tile([128, QW], F32, tag=f"p{_palt[0]}")
                _palt[0] ^= 1
                for kc in range(CP):
                    nc.tensor.matmul(
                        ps[:],
                        attnT[:, kc * T + tp * 128 : kc * T + tp * 128 + 128],
                        wot[:, kc * D + ob * QW : (kc * D) + (ob + 1) * QW],
                        start=(kc == 0),
                        stop=(kc == CP - 1),
                    )
                yt = ystage.tile([128, QW], BF16, tag="yt")
                if tail:
                    nc.scalar.copy(yt[:], ps[:])
                else:
                    nc.vector.tensor_copy(yt[:], ps[:])
                nc.sync.dma_start(
                    y[tp * 128 : (tp + 1) * 128, ob * QW : (ob + 1) * QW], yt[:]
                )

            pts = {}
            ost = {}
            SEGS = [(0, 0), (0, 1), (0, 2), (0, 3), (1, 0), (1, 1), (1, 2), (1, 3)]

            def seg_scores(seg, i):
                j, q = SEGS[seg]
                t1o = q * QW
                s = ps_s.tile([128, 2 * QW], F32, tag=f"s{i % 2}")
                for h in range(2):  # PE tiles T0 / T8, concurrent
                    hp = h * 64
                    nc.tensor.matmul(
                        s[:, h * QW : (h + 1) * QW],
                        kTt[hp : hp + 64, j * T + i * 128 : j * T + i * 128 + 128],
                        qTt[hp : hp + 64, j * T + t1o : j * T + t1o + QW],
                        start=True,
                        stop=True,
                    )
                pt = pexp.tile([128, 2 * QW], BF16, tag="pt")
                nc.scalar.activation(pt[:], s[:], EXP, scale=0.125)
                pts[(seg, i)] = pt

            def seg_pv(seg, i):
                j, q = SEGS[seg]
                if i == 0:
                    ost[seg] = (
                        ps_o.tile([128, QW], F32, tag="o0", name=f"o0_{seg}"),
                        ps_o.tile([128, QW], F32, tag="o1", name=f"o1_{seg}"),
                    )
                o0, o1 = ost[seg]
                pt = pts.pop((seg, i))
                for hh, o_ps in ((2 * j, o0), (2 * j + 1, o1)):
                    nc.tensor.matmul(
                        o_ps[:],
                        vt[:, i * VROW + VW * hh : i * VROW + VW * hh + 128],
                        pt[:, (hh % 2) * QW : (hh % 2) * QW + QW],
                        start=(i == 0),
                        stop=(i == TP - 1),
                    )

            def seg_norm(seg, final=False):
                j, q = SEGS[seg]
                t1o = q * QW
                o0, o1 = ost.pop(seg)
                # eager PSUM evacuation (copies free o0/o1 for the next
                # segment), both heads' reciprocals packed into one [1,1024]
                # tile so a SINGLE gpsimd broadcast serves both muls (the
                # second broadcast used to eat a ~1.2us gpsimd DRAIN)
                rt = small.tile([1, 2 * QW], F32, tag="rt")
                Rt = small.tile([64, 2 * QW], F32, tag="Rt")
                srcs = []
                for hh, o_ps in ((2 * j, o0), (2 * j + 1, o1)):
                    scr = small.tile([1, QW], F32, tag=f"scr{hh % 2}")
                    if final:
                        # tail: no successor needs o0/o1 -> no evacuation;
                        # denom copy on the idle scalar queue, multiply
                        # straight out of PSUM
                        nc.scalar.copy(scr[:], o_ps[64:65, :])
                        srcs.append(o_ps[0:64, :])
                    else:
                        orw = small.tile([64, QW], F32, tag=f"or{hh % 2}")
                        nc.vector.tensor_copy(orw[:], o_ps[0:64, :])
                        nc.vector.tensor_copy(scr[:], o_ps[64:65, :])
                        srcs.append(orw[:])
                    nc.vector.reciprocal_approx_fast(
                        rt[:, (hh % 2) * QW : (hh % 2) * QW + QW], scr[:]
                    )
                nc.gpsimd.partition_broadcast(Rt[:], rt[:])
                for hp, src0 in enumerate(srcs):
                    nc.vector.tensor_mul(
                        attnT[hp * 64 : hp * 64 + 64, j * T + t1o : j * T + t1o + QW],
                        src0,
                        Rt[:, hp * QW : hp * QW + QW],
                    )

            # --- pre-phase: pair-0 Q/K, kd-OUTER across all 8 PSUM banks so
            # each xT chunk is consumed as its DMA lands ---
            s0 = ps_s.tile([128, 2 * QW], F32, tag="s0")
            s1 = ps_s.tile([128, 2 * QW], F32, tag="s1")
            o0 = ps_o.tile([128, QW], F32, tag="o0")
            o1 = ps_o.tile([128, QW], F32, tag="o1")
            p0 = ps_p.tile([128, QW], F32, tag="p0")
            p1 = ps_p.tile([128, QW], F32, tag="p1")
            _palt[0] = 0  # p0/p1 consumed above; keep alternation in sync
            pre = [  # (psum slice, weight sbuf, dest sbuf, t-block)
                (s0[:, 0:QW], wqt, qTt, 0),
                (s0[:, QW : 2 * QW], wkt, kTt, 0),
                (s1[:, 0:QW], wqt, qTt, 1),
                (s1[:, QW : 2 * QW], wkt, kTt, 1),
                (o0[:], wqt, qTt, 2),
                (o1[:], wkt, kTt, 2),
                (p0[:], wqt, qTt, 3),
                (p1[:], wkt, kTt, 3),
            ]
            def pre_mm(u, kd):
                ps_sl, wsb, _dst, tb = pre[u]
                nc.tensor.matmul(
                    ps_sl,
                    wsb[:, kd * CH : kd * CH + 128],
                    xTt[:, kd * T + tb * QW : kd * T + (tb + 1) * QW],
                    start=(kd == 0),
                    stop=(kd == KD - 1),
                )

            # kd 0-3: all units, kd-outer (paced by xT chunks + wk half 1);
            # kd 4-7: Q units kd-outer (paced by xT), then K units
            # unit-major, K-tb0 first, as soon as wk half 2 lands
            for kd in range(4):
                for u in range(8):
                    pre_mm(u, kd)
            for kd in range(4, 8):
                for u in (0, 2, 4, 6):
                    pre_mm(u, kd)
            for u in (1, 3, 5, 7):
                for kd in range(4, 8):
                    pre_mm(u, kd)
            # evacuate the s banks (scores(0)/(1) reuse them immediately)
            # and p banks (V units need them); the o pair isn't reused
            # until pv(0,0) so its copies defer behind the first V units,
            # letting their copies reach the vector queue sooner
            for idx in (0, 1, 2, 3, 6, 7):
                ps_sl, _wsb, dst, tb = pre[idx]
                nc.vector.tensor_copy(dst[:, tb * QW : (tb + 1) * QW], ps_sl)
            proj_v_unit(0)
            proj_v_unit(1)
            for idx in (4, 5):
                ps_sl, _wsb, dst, tb = pre[idx]
                nc.vector.tensor_copy(dst[:, tb * QW : (tb + 1) * QW], ps_sl)

            # --- filler schedule: V chunk tp before pv(i=tp) in segment 0;
            # pair-1 Q/K before segment 4; wo(q) after segment 4+q, kept off
            # the first blocks of each segment (norm still completing) ---
            E = []
            V = [lambda tp=tp: proj_v_unit(tp) for tp in range(16)]
            qk1 = []
            for t8 in range(8):
                qk1.append(lambda t8=t8: proj_qk_unit(1, qTt, wqt, t8))
                qk1.append(lambda t8=t8: proj_qk_unit(1, kTt, wkt, t8))

            def wo_fills(q):
                w = []
                for tp in range(q * 4, q * 4 + 4):
                    w.append(lambda tp=tp: wo_unit(tp, 0))
                    w.append(lambda tp=tp: wo_unit(tp, 1))
                return [E, E, w[0:2], w[2:4], w[4:6], w[6:8]]

            slots = (
                [E, V[2:4], V[4:6], V[6:8], V[8:10], V[10:12], V[12:14], V[14:16]]
                + [qk1[0:1], qk1[1:2], qk1[2:3], E, E, E, E, E]
                + [qk1[3:4], qk1[5:6], qk1[7:8], E, E, E, E, E]
                + [qk1[9:10], qk1[11:12], qk1[13:14], E, E, E, E, E]
                + [qk1[15:16], qk1[4:5], qk1[6:7], E, E, E, E, E]
                + [qk1[8:9], qk1[10:11], E, E] + wo_fills(0)[2:]
                + [qk1[12:13], qk1[14:15], E, E] + wo_fills(1)[2:]
                + [E, E] + wo_fills(2)[2:] + [E, E]
            )

            # --- one continuous software-pipelined stream over all 8
            # segments: scores of block gb, pv of block gb-1 (crossing
            # segment boundaries), fillers; norm emitted as soon as a
            # segment's last pv is down ---
            for gb in range(64):
                seg, ib = divmod(gb, 8)
                seg_scores(seg, 2 * ib)
                seg_scores(seg, 2 * ib + 1)
                b = gb - 1
                if b >= 0:
                    bs, bb = divmod(b, 8)
                    if bb == 0:
                        pv_blocks = []  # delayed: previous norm still
                        # evacuating o0/o1; scores+fillers cover the gap
                    elif bb == 1:
                        pv_blocks = [b - 1, b]
                    else:
                        pv_blocks = [b]
                    for pb in pv_blocks:
                        pseg, pib = divmod(pb, 8)
                        seg_pv(pseg, 2 * pib)
                        seg_pv(pseg, 2 * pib + 1)
                        if pib == 7:
                            seg_norm(pseg)
                for f in slots[gb]:
                    f()
            seg_pv(7, 14)
            seg_pv(7, 15)
            seg_norm(7, final=True)
            for tp in range(12, 16):
                wo_unit(tp, 0, tail=True)
                wo_unit(tp, 1, tail=True)
    nc.compile()
    return nc


def kernel(x, wq, wk, wv, wo, trace=False):
    global _cached_nc
    if _cached_nc is None:
        _cached_nc = _build()
    nc = _cached_nc

    ones = np.ones((NH * TP, 128), ml_dtypes.bfloat16)
    x = np.asarray(x, dtype=np.float32)
    wq = np.asarray(wq, dtype=np.float32)
    wk = np.asarray(wk, dtype=np.float32)
    wv = np.asarray(wv, dtype=np.float32)
    wo = np.asarray(wo, dtype=np.float32)

    in_maps = []
    for c in range(8):
        b, g = c // 4, c % 4
        cs = slice(g * CH, (g + 1) * CH)
        in_maps.append(
            {
                "xT": _wlayout(np.ascontiguousarray(x[b].T)).astype(
                    ml_dtypes.bfloat16
                ),
                "wq": _wlayout(wq[:, cs]).astype(ml_dtypes.bfloat16),
                "wk": _wlayout(wk[:, cs]).astype(ml_dtypes.bfloat16),
                "wv": _wlayout(wv[:, cs]).astype(ml_dtypes.bfloat16),
                "wo": _wlayout(wo[cs, :]).astype(ml_dtypes.bfloat16),
                "ones": ones,
            }
        )

    # the device intermittently drops input DMAs after a prior crash,
    # yielding inf/garbage; detect the signature and retry (healthy runs
    # have |y| ~ O(1))
    for _attempt in range(4):
        res = run_bass_kernel_spmd(
            nc, in_maps, core_ids=list(range(8)), trace=trace
        )
        out = np.zeros((B, T, D), np.float32)
        for c in range(8):
            b = c // 4
            out[b] += np.asarray(res.results[c]["y"], dtype=np.float32)
        if np.isfinite(out).all() and np.abs(out).max() < 1e3:
            break
    if trace:
        kernel.last_results = res
    return out


# revision 48
# speedup vs baseline: 1.1869x; 1.1869x over previous
"""Multi-head self-attention on 8 Trainium2 NeuronCores.

Sharding: batch (2) x head-groups (4 groups of 4 heads) -> 8 cores.
Per core: x[b] @ wq/wk/wv column slices (256 ch), 4 heads of attention,
row-parallel wo -> partial [2048, 1024] output; host sums the 4 group
partials per batch (the unshard step for row-parallel wo).

v3 dataflow (head-pair packing + PE row tiling, all-bf16 matmuls):
  qT/kT [128, 2*2048] bf16: pair j at cols j*T; head 2j on partitions
        0-63, head 2j+1 on partitions 64-127. Score matmuls contract
        K=64 from base partition 0 / 64 -> they land on PE array tiles
        T0/T8 (64x128 row-tiled mode) and stream CONCURRENTLY (verified
        on HW: the pair overlaps fully), so a head pair's scores cost
        one matmul.
  V     interleaved [2048 t, 4*65+pad] bf16 with a ones column per head
        (PV emits the softmax denominator as PSUM row 64 for free).
  s     PSUM [128 t2, 1024] = both heads' 512-wide t1 quarter,
        ping-pong (s0/s1); ONE exp ACTIVATE [128,1024] per i straight
        off PSUM (scores~N(0,1), no max-subtraction), bf16 out. PV runs
        one i-pair behind exp (software pipeline) so the in-order
        tensor queue never stalls on the exp latency.
  PSUM  s0+s1 (4 banks) + o0+o1 ([128,512] accumulators, 2 banks) +
        p0+p1 (2 spare banks for projection/wo units).
  sched pair-0 Q/K projections run kd-OUTER across all 8 PSUM banks so
        they pipeline with the xT input DMA; V projection, pair-1 Q/K,
        and per-quarter wo units are emitted as fillers inside the
        attention i-loops, deadline-ordered, hiding them in the slack
        between the exp stream (ScalarE, ~1us/iter) and the attention
        matmuls. attnT/wo/y all bf16 (fp32 matmul runs 3-4x slower on
        the PE; bf16 keeps rel err ~7e-3 << 2e-2).
Measured: see test.py.
"""

import sys

sys.path.insert(0, "/opt/trn_rl_repo")

import numpy as np
import ml_dtypes
import concourse.bass as bass
import concourse.mybir as mybir
import concourse.tile as tile
from concourse import bacc
from concourse.bass_utils import run_bass_kernel_spmd

B, T, D = 2, 2048, 1024
NH = 4  # heads per core
HD = 64  # head dim
CH = NH * HD  # 256 channels per core
KD = D // 128  # 8 k-ptiles
CP = CH // 128  # 2 c-ptiles (head pairs)
TP = T // 128  # 16 t-ptiles
QW = 512  # t1 quarter width
NQ = T // QW  # 4 quarters
VW = HD + 1  # 65: v columns + ones column
VROW = NH * VW  # 260

F32 = mybir.dt.float32
EXP = mybir.ActivationFunctionType.Exp
BF16 = mybir.dt.bfloat16

_cached_nc = None


def _wlayout(w):
    """[G*128, C] -> [128, G*C]: host-side relayout matching the SBUF tiles
    so the weight DMAs are fully contiguous."""
    g = w.shape[0] // 128
    return np.ascontiguousarray(
        w.reshape(g, 128, w.shape[1]).transpose(1, 0, 2).reshape(128, -1)
    )


def _build():
    nc = bacc.Bacc(None, target_bir_lowering=False)
    xT = nc.dram_tensor("xT", [128, KD * T], BF16, kind="ExternalInput")
    ones = nc.dram_tensor("ones", [NH * TP, 128], BF16, kind="ExternalInput")
    wq = nc.dram_tensor("wq", [128, KD * CH], BF16, kind="ExternalInput")
    wk = nc.dram_tensor("wk", [128, KD * CH], BF16, kind="ExternalInput")
    wv = nc.dram_tensor("wv", [128, KD * CH], BF16, kind="ExternalInput")
    wo = nc.dram_tensor("wo", [128, CP * D], BF16, kind="ExternalInput")
    y = nc.dram_tensor("y", [T, D], BF16, kind="ExternalOutput")

    with tile.TileContext(nc) as tc:
        with (
            tc.tile_pool(name="sb", bufs=1) as sb,
            tc.tile_pool(name="pexp", bufs=10) as pexp,
            tc.tile_pool(name="small", bufs=3) as small,
            tc.tile_pool(name="ystage", bufs=6) as ystage,
            tc.tile_pool(name="ps_s", bufs=1, space="PSUM") as ps_s,
            tc.tile_pool(name="ps_o", bufs=1, space="PSUM") as ps_o,
            tc.tile_pool(name="ps_p", bufs=1, space="PSUM") as ps_p,
        ):
            wot = sb.tile([128, CP * D], BF16)
            qTt = sb.tile([128, CP * T], BF16)
            kTt = sb.tile([128, CP * T], BF16)
            vt = sb.tile([128, TP * VROW + 64], BF16)
            attnT = sb.tile([128, CP * T], BF16)
            wqt = sb.tile([128, KD * CH], BF16)
            wkt = sb.tile([128, KD * CH], BF16)
            wvt = sb.tile([128, KD * CH], BF16)
            xTt = sb.tile([128, KD * T], BF16)

            # --- input DMAs, ordered so the qk pre-phase streams with xT ---
            for wt_sb, wt_dr in ((wqt, wq), (wkt, wk)):
                nc.sync.dma_start(wt_sb[:], wt_dr[:])
            for kd in range(KD):  # per-chunk gating so the pre-phase streams
                nc.sync.dma_start(
                    xTt[:, kd * T : (kd + 1) * T], xT[:, kd * T : (kd + 1) * T]
                )
            nc.sync.dma_start(wvt[:], wv[:])
            nc.sync.dma_start(wot[:], wo[:])
            # ones columns of vt: offsets 64 + 65*k, k = 0..NH*TP-1
            nc.sync.dma_start(
                bass.AP(vt.tensor, HD, [[TP * VROW + 64, 128], [VW, NH * TP]]),
                ones.rearrange("k p -> p k"),
            )
            # init the 64-col pad tail (read as junk M-padding by the last
            # head's PV lhsT; must not be uninitialized SBUF)
            nc.sync.dma_start(
                vt[:, TP * VROW : TP * VROW + 64],
                ones.rearrange("k p -> p k"),
            )

            # warm-up: pull the one-time ACT table load (~2.7us for the
            # exp set), custom-DVE reciprocal ucode, and gpsimd broadcast
            # setup into the DMA-wait dead window at t~0 instead of paying
            # them on the first real use mid-stream
            wsc = sb.tile([1, 8], F32)
            nc.vector.memset(wsc[:], 1.0)
            wex = sb.tile([1, 8], BF16)
            nc.scalar.activation(wex[:], wsc[:], EXP, scale=0.125)
            wrc = sb.tile([1, 8], F32)
            nc.vector.reciprocal_approx_fast(wrc[:], wsc[:])
            wbc = sb.tile([64, 8], F32)
            nc.gpsimd.partition_broadcast(wbc[:], wrc[:])

            _palt = [0]

            def proj_qk_unit(cp, dst, wsb, t8):
                # 256-wide (half t-block) so a unit is a ~1us filler piece
                ps = ps_p.tile([128, QW], F32, tag=f"p{_palt[0]}")
                _palt[0] ^= 1
                off = t8 * 256
                for kd in range(KD):
                    nc.tensor.matmul(
                        ps[:, 0:256],
                        wsb[:, kd * CH + cp * 128 : kd * CH + cp * 128 + 128],
                        xTt[:, kd * T + off : kd * T + off + 256],
                        start=(kd == 0),
                        stop=(kd == KD - 1),
                    )
                nc.vector.tensor_copy(
                    dst[:, cp * T + off : cp * T + off + 256], ps[:, 0:256]
                )

            def proj_v_unit(tp):  # one t2 chunk
                ps = ps_p.tile([128, QW], F32, tag=f"p{_palt[0]}")
                _palt[0] ^= 1
                for kd in range(KD):
                    nc.tensor.matmul(
                        ps[:, 0:CH],
                        xTt[:, kd * T + tp * 128 : kd * T + tp * 128 + 128],
                        wvt[:, kd * CH : (kd + 1) * CH],
                        start=(kd == 0),
                        stop=(kd == KD - 1),
                    )
                nc.vector.tensor_copy(
                    bass.AP(
                        vt.tensor,
                        tp * VROW,
                        [[TP * VROW + 64, 128], [VW, NH], [1, HD]],
                    ),
                    ps[:, 0:CH].rearrange("p (h c) -> p h c", h=NH),
                )

            def wo_unit(tp, ob, tail=False):  # y tile [128 t1, 512 d]
                ps = ps_p.tile([128, QW], F32, tag=f"p{_palt[0]}")
                _palt[0] ^= 1
                for kc in range(CP):
                    nc.tensor.matmul(
                        ps[:],
                        attnT[:, kc * T + tp * 128 : kc * T + tp * 128 + 128],
                        wot[:, kc * D + ob * QW : (kc * D) + (ob + 1) * QW],
                        start=(kc == 0),
                        stop=(kc == CP - 1),
                    )
                yt = ystage.tile([128, QW], BF16, tag="yt")
                if tail:
                    nc.scalar.copy(yt[:], ps[:])
                else:
                    nc.vector.tensor_copy(yt[:], ps[:])
                nc.sync.dma_start(
                    y[tp * 128 : (tp + 1) * 128, ob * QW : (ob + 1) * QW], yt[:]
                )

            pts = {}
            ost = {}
            SEGS = [(0, 0), (0, 1), (0, 2), (0, 3), (1, 0), (1, 1), (1, 2), (1, 3)]

            def seg_scores(seg, i):
                j, q = SEGS[seg]
                t1o = q * QW
                s = ps_s.tile([128, 2 * QW], F32, tag=f"s{i % 2}")
                for h in range(2):  # PE tiles T0 / T8, concurrent
                    hp = h * 64
                    nc.tensor.matmul(
                        s[:, h * QW : (h + 1) * QW],
                        kTt[hp : hp + 64, j * T + i * 128 : j * T + i * 128 + 128],
                        qTt[hp : hp + 64, j * T + t1o : j * T + t1o + QW],
                        start=True,
                        stop=True,
                    )
                pt = pexp.tile([128, 2 * QW], BF16, tag="pt")
                nc.scalar.activation(pt[:], s[:], EXP, scale=0.125)
                pts[(seg, i)] = pt

            def seg_pv(seg, i):
                j, q = SEGS[seg]
                if i == 0:
                    ost[seg] = (
                        ps_o.tile([128, QW], F32, tag="o0", name=f"o0_{seg}"),
                        ps_o.tile([128, QW], F32, tag="o1", name=f"o1_{seg}"),
                    )
                o0, o1 = ost[seg]
                pt = pts.pop((seg, i))
                for hh, o_ps in ((2 * j, o0), (2 * j + 1, o1)):
                    nc.tensor.matmul(
                        o_ps[:],
                        vt[:, i * VROW + VW * hh : i * VROW + VW * hh + 128],
                        pt[:, (hh % 2) * QW : (hh % 2) * QW + QW],
                        start=(i == 0),
                        stop=(i == TP - 1),
                    )

            def seg_norm(seg, final=False):
                j, q = SEGS[seg]
                t1o = q * QW
                o0, o1 = ost.pop(seg)
                # eager PSUM evacuation (copies free o0/o1 for the next
                # segment), both heads' reciprocals packed into one [1,1024]
                # tile so a SINGLE gpsimd broadcast serves both muls (the
                # second broadcast used to eat a ~1.2us gpsimd DRAIN)
                rt = small.tile([1, 2 * QW], F32, tag="rt")
                Rt = small.tile([64, 2 * QW], F32, tag="Rt")
                srcs = []
                for hh, o_ps in ((2 * j, o0), (2 * j + 1, o1)):
                    scr = small.tile([1, QW], F32, tag=f"scr{hh % 2}")
                    if final:
                        # tail: no successor needs o0/o1 -> no evacuation;
                        # denom copy on the idle scalar queue, multiply
                        # straight out of PSUM
                        nc.scalar.copy(scr[:], o_ps[64:65, :])
                        srcs.append(o_ps[0:64, :])
                    else:
                        orw = small.tile([64, QW], F32, tag=f"or{hh % 2}")
                        nc.vector.tensor_copy(orw[:], o_ps[0:64, :])
                        nc.vector.tensor_copy(scr[:], o_ps[64:65, :])
                        srcs.append(orw[:])
                    nc.vector.reciprocal_approx_fast(
                        rt[:, (hh % 2) * QW : (hh % 2) * QW + QW], scr[:]
                    )
                nc.gpsimd.partition_broadcast(Rt[:], rt[:])
                for hp, src0 in enumerate(srcs):
                    nc.vector.tensor_mul(
                        attnT[hp * 64 : hp * 64 + 64, j * T + t1o : j * T + t1o + QW],
                        src0,
                        Rt[:, hp * QW : hp * QW + QW],
                    )

            # --- pre-phase: pair-0 Q/K, kd-OUTER across all 8 PSUM banks so
            # each xT chunk is consumed as its DMA lands ---
            s0 = ps_s.tile([128, 2 * QW], F32, tag="s0")
            s1 = ps_s.tile([128, 2 * QW], F32, tag="s1")
            o0 = ps_o.tile([128, QW], F32, tag="o0")
            o1 = ps_o.tile([128, QW], F32, tag="o1")
            p0 = ps_p.tile([128, QW], F32, tag="p0")
            p1 = ps_p.tile([128, QW], F32, tag="p1")
            _palt[0] = 0  # p0/p1 consumed above; keep alternation in sync
            pre = [  # (psum slice, weight sbuf, dest sbuf, t-block)
                (s0[:, 0:QW], wqt, qTt, 0),
                (s0[:, QW : 2 * QW], wkt, kTt, 0),
                (s1[:, 0:QW], wqt, qTt, 1),
                (s1[:, QW : 2 * QW], wkt, kTt, 1),
                (o0[:], wqt, qTt, 2),
                (o1[:], wkt, kTt, 2),
                (p0[:], wqt, qTt, 3),
                (p1[:], wkt, kTt, 3),
            ]
            for kd in range(KD):
                for ps_sl, wsb, _dst, tb in pre:
                    nc.tensor.matmul(
                        ps_sl,
                        wsb[:, kd * CH : kd * CH + 128],
                        xTt[:, kd * T + tb * QW : kd * T + (tb + 1) * QW],
                        start=(kd == 0),
                        stop=(kd == KD - 1),
                    )
            # evacuate the s banks (scores(0)/(1) reuse them immediately)
            # and p banks (V units need them); the o pair isn't reused
            # until pv(0,0) so its copies defer behind the first V units,
            # letting their copies reach the vector queue sooner
            for idx in (0, 1, 2, 3, 6, 7):
                ps_sl, _wsb, dst, tb = pre[idx]
                nc.vector.tensor_copy(dst[:, tb * QW : (tb + 1) * QW], ps_sl)
            proj_v_unit(0)
            proj_v_unit(1)
            for idx in (4, 5):
                ps_sl, _wsb, dst, tb = pre[idx]
                nc.vector.tensor_copy(dst[:, tb * QW : (tb + 1) * QW], ps_sl)

            # --- filler schedule: V chunk tp before pv(i=tp) in segment 0;
            # pair-1 Q/K before segment 4; wo(q) after segment 4+q, kept off
            # the first blocks of each segment (norm still completing) ---
            E = []
            V = [lambda tp=tp: proj_v_unit(tp) for tp in range(16)]
            qk1 = []
            for t8 in range(8):
                qk1.append(lambda t8=t8: proj_qk_unit(1, qTt, wqt, t8))
                qk1.append(lambda t8=t8: proj_qk_unit(1, kTt, wkt, t8))

            def wo_fills(q):
                w = []
                for tp in range(q * 4, q * 4 + 4):
                    w.append(lambda tp=tp: wo_unit(tp, 0))
                    w.append(lambda tp=tp: wo_unit(tp, 1))
                return [E, E, w[0:2], w[2:4], w[4:6], w[6:8]]

            slots = (
                [E, V[2:4], V[4:6], V[6:8], V[8:10], V[10:12], V[12:14], V[14:16]]
                + [qk1[0:1], qk1[1:2], qk1[2:3], E, E, E, E, E]
                + [qk1[3:4], qk1[5:6], qk1[7:8], E, E, E, E, E]
                + [qk1[9:10], qk1[11:12], qk1[13:14], E, E, E, E, E]
                + [qk1[15:16], qk1[4:5], qk1[6:7], E, E, E, E, E]
                + [qk1[8:9], qk1[10:11], E, E] + wo_fills(0)[2:]
                + [qk1[12:13], qk1[14:15], E, E] + wo_fills(1)[2:]
                + [E, E] + wo_fills(2)[2:] + [E, E]
            )

            # --- one continuous software-pipelined stream over all 8
            # segments: scores of block gb, pv of block gb-1 (crossing
            # segment boundaries), fillers; norm emitted as soon as a
            # segment's last pv is down ---
            for gb in range(64):
                seg, ib = divmod(gb, 8)
                seg_scores(seg, 2 * ib)
                seg_scores(seg, 2 * ib + 1)
                b = gb - 1
                if b >= 0:
                    bs, bb = divmod(b, 8)
                    if bb == 0:
                        pv_blocks = []  # delayed: previous norm still
                        # evacuating o0/o1; scores+fillers cover the gap
                    elif bb == 1:
                        pv_blocks = [b - 1, b]
                    else:
                        pv_blocks = [b]
                    for pb in pv_blocks:
                        pseg, pib = divmod(pb, 8)
                        seg_pv(pseg, 2 * pib)
                        seg_pv(pseg, 2 * pib + 1)
                        if pib == 7:
                            seg_norm(pseg)
                for f in slots[gb]:
                    f()
            seg_pv(7, 14)
            seg_pv(7, 15)
            seg_norm(7, final=True)
            for tp in range(12, 16):
                wo_unit(tp, 0, tail=True)
                wo_unit(tp, 1, tail=True)
    nc.compile()
    return nc


def kernel(x, wq, wk, wv, wo, trace=False):
    global _cached_nc
    if _cached_nc is None:
        _cached_nc = _build()
    nc = _cached_nc

    ones = np.ones((NH * TP, 128), ml_dtypes.bfloat16)
    x = np.asarray(x, dtype=np.float32)
    wq = np.asarray(wq, dtype=np.float32)
    wk = np.asarray(wk, dtype=np.float32)
    wv = np.asarray(wv, dtype=np.float32)
    wo = np.asarray(wo, dtype=np.float32)

    in_maps = []
    for c in range(8):
        b, g = c // 4, c % 4
        cs = slice(g * CH, (g + 1) * CH)
        in_maps.append(
            {
                "xT": _wlayout(np.ascontiguousarray(x[b].T)).astype(
                    ml_dtypes.bfloat16
                ),
                "wq": _wlayout(wq[:, cs]).astype(ml_dtypes.bfloat16),
                "wk": _wlayout(wk[:, cs]).astype(ml_dtypes.bfloat16),
                "wv": _wlayout(wv[:, cs]).astype(ml_dtypes.bfloat16),
                "wo": _wlayout(wo[cs, :]).astype(ml_dtypes.bfloat16),
                "ones": ones,
            }
        )

    # the device intermittently drops input DMAs after a prior crash,
    # yielding inf/garbage; detect the signature and retry (healthy runs
    # have |y| ~ O(1))
    for _attempt in range(4):
        res = run_bass_kernel_spmd(
            nc, in_maps, core_ids=list(range(8)), trace=trace
        )
        out = np.zeros((B, T, D), np.float32)
        for c in range(8):
            b = c // 4
            out[b] += np.asarray(res.results[c]["y"], dtype=np.float32)
        if np.isfinite(out).all() and np.abs(out).max() < 1e3:
            break
    if trace:
        kernel.last_results = res
    return out
